# revision 4
# baseline (speedup 1.0000x reference)
"""BPR loss kernel for Trainium2 (8 NeuronCores, SPMD data-parallel).

Problem:
    predict: (4096, 100000) f32, pos_idx/neg_idx: (4096, 50) int
    loss = sum_b -mean_k logsigmoid(predict[b, pos_idx[b,k]] - predict[b, neg_idx[b,k]])

Strategy (per core, 512 rows = 25600 (pos, neg) pairs):
    - the loss is a flat sum of -logsigmoid(pos - neg) over pairs, so pair
      placement is arbitrary: the host sorts pairs by pos address and deals
      each contiguous 1/16 of the sorted span to one SDMA engine's 8
      partitions (round-robin), tiled [posA|negA|posB|negB] x 100 cols per
      partition. Every engine's pos-descriptor stream then walks one
      ascending HBM window, which cuts the gather drains from ~4.5us to
      ~3.1us total and removes most of the 20us+ outlier runs (the drains
      are device-HBM-transaction-bound across all 8 cores).
    - idx half A loads alone on the SP HWDGE ring (its completion receipt,
      ~2.6us after exec, opens the critical path); half B rides the ACT ring
      behind the exp/ln table pre-warm (PWP ~1.3us), arriving just before
      gather B's descriptor generation needs it.
    - two SWDGE indirect DMAs gather 2x25600 scalars (descgen ~1.1us each,
      count-independent); a small dummy indirect DMA first pulls the Q7
      indirect-copy ucode (cold start ~2.4us) while the idx tiles load.
    - -logsigmoid(d) = ln(1 + exp(-d)) per chunk: DVE subtract, ACT Exp,
      ACT Ln with bias=1.0 and fused per-partition row-sum (accum_out).
      Exp and Ln share one ACT table set (natural_log_exp_and_others).
      ACT passes are overhead-dominated (~290ns fixed + ~90ns/100 cols), so
      2 chunks balance ACT serialization against drain overlap.
    - PE dots each 128x1 partial with a ones vector into PSUM as soon as its
      ln completes (chunk A's matmul hides under drain B; PE is pre-warmed
      by a dummy matmul during the DMA phase); after ln B only one small
      matmul + DVE PSUM->SBUF copy + an 8B SP store remain (single
      descriptor; a 128-partition store pays ~7us in per-engine HBM
      completion receipts).
    Host sums the 8x2 scalars and divides by K.

Fixed costs bound this kernel: ~7.2us runtime+bass preamble before the first
kernel instruction and ~2.2us closing barrier/postamble (an empty kernel
measures ~10.5us end-to-end); the remaining ~9us of body is idx receipt +
descgen + HBM-bound drains + one chunk of compute + the reduce/store tail.

Rejected structural alternatives (all verified to fail or not help):
  - offsets AP in DRAM (skip the idx SBUF load): walrus generateDynamicDMA
    rejects non-SBUF offsets
  - SBUF-resident ExternalInput for idx (runtime preload): walrus
    assign64bitAddr rejects SB-space IO tensors
  - ActivationFunctionType.Softplus (1 ACT pass instead of 2): the
    compiler's act table set has no softplus entry
  - bf16 intermediates: ACT passes are overhead-bound, no speedup
  - dma_gather: requires int16 indices and 256B elements

Raw bass (no Tile): the Tile tail drain accumulates >4 sem waits on one
instruction, which the walrus codegen rejects ("Too many sync wait commands").
"""

import numpy as np

import concourse.bass as bass
from concourse import mybir
from concourse.bass_utils import run_bass_kernel_spmd

B, N, K = 4096, 100000, 50
NCORES = 8
RB = B // NCORES          # 512 rows per core
P = 128                   # SBUF partitions
PAIRS = RB * K            # 25600 pairs per core
TC = PAIRS // P           # 200 = pair-columns per partition (both chunks)
CA = 140                  # chunk A pair-cols per partition (hidden under drain B)
CB = TC - CA              # 60 = chunk B pair-cols (post-drain tail work)
HC = TC // 2              # legacy symmetric size (kept for test.py fallback)

_NC_CACHE = None


def build_bass():
    nc = bass.Bass(monotonic_sem_count=0)
    predict = nc.declare_dram_parameter(
        "predict", [RB * N, 1], mybir.dt.float32, isOutput=False
    )
    idx = nc.declare_dram_parameter("idx", [P, 2 * TC], mybir.dt.int32, isOutput=False)
    out = nc.declare_dram_parameter("out", [1, 2], mybir.dt.float32, isOutput=True)

    f32 = mybir.dt.float32
    AF = mybir.ActivationFunctionType
    ones = nc.const_aps.aps[(f32, 1.0)]   # [128, 1], memset in preamble
    zero = nc.const_aps.aps[(f32, 0.0)]   # [128, 1]

    WA = 2 * CA  # 280 = idx/vals cols of chunk A (pos|neg)

    from contextlib import ExitStack

    with ExitStack() as ctx:
        ec = ctx.enter_context
        idx_t = ec(nc.sbuf_tensor([P, 2 * TC], mybir.dt.int32))
        vals = ec(nc.sbuf_tensor([P, 2 * TC], f32))
        d = ec(nc.sbuf_tensor([P, TC], f32))
        e = ec(nc.sbuf_tensor([P, TC], f32))
        act_out = ec(nc.sbuf_tensor([P, TC], f32))
        part = ec(nc.sbuf_tensor([P, 2], f32))
        dummy = ec(nc.sbuf_tensor([P, 1], f32))
        scalar_out = ec(nc.sbuf_tensor([1, 2], f32))
        warm_out = ec(nc.sbuf_tensor([P, 2], f32))
        psum_s = ec(nc.psum_tensor([1, 2], f32))
        psum_w = ec(nc.psum_tensor([1, 1], f32))
        s_warm = ec(nc.semaphore("s_warm"))
        s_out = ec(nc.semaphore("s_out"))
        s_i1 = ec(nc.semaphore("s_i1"))
        s_i1b = ec(nc.semaphore("s_i1b"))
        s_i2 = ec(nc.semaphore("s_i2"))
        s_g1 = ec(nc.semaphore("s_g1"))
        s_g2 = ec(nc.semaphore("s_g2"))
        sv = ec(nc.semaphore("sv"))     # DVE subtracts
        se = ec(nc.semaphore("se"))     # ACT exps
        sl = ec(nc.semaphore("sl"))     # ACT lns
        sm = ec(nc.semaphore("sm"))     # PE matmul
        sc = ec(nc.semaphore("sc"))     # DVE psum copy
        block = ec(nc.Block())

        @block.sync
        def _(sync):
            # idx chunk-A pos half on the SP HWDGE ring; its completion
            # receipt (~1.7-2.9us after exec) opens the critical path
            sync.dma_start(out=idx_t[:, :CA], in_=idx[:, :CA]).then_inc(s_i1, 16)
            sync.wait_ge(sc, 1)
            sync.dma_start(out=out[:], in_=scalar_out[:]).then_inc(s_out, 16)
            # no wait on s_out: the 8B HBM write's completion receipt costs
            # ~2.2us; the runtime quiesces DMA rings before results are read

        @block.scalar
        def _(scalar):
            # ACT ring: chunk-A neg half of idx, then the exp/ln table
            # pre-warm (PWP), then idx B (needed only at gather-B descgen)
            scalar.dma_start(out=idx_t[:, CA:WA], in_=idx[:, CA:WA]).then_inc(s_i1b, 16)
            nc.scalar.activation(out=dummy[:], in_=zero, func=AF.Exp)
            scalar.dma_start(out=idx_t[:, WA:], in_=idx[:, WA:]).then_inc(s_i2, 16)
            # -logsigmoid(pos-neg) = ln(1 + exp(neg-pos)): Exp pass then Ln
            # with bias=1.0 and fused per-partition row-sum (accum_out)
            scalar.wait_ge(sv, 1)
            nc.scalar.activation(out=e[:, :CA], in_=d[:, :CA], func=AF.Exp).then_inc(
                se, 1
            )
            scalar.wait_ge(se, 1)
            nc.scalar.activation(
                out=act_out[:, :CA],
                in_=e[:, :CA],
                func=AF.Ln,
                bias=1.0,
                accum_out=part[:, 0:1],
            ).then_inc(sl, 1)
            scalar.wait_ge(sv, 2)
            nc.scalar.activation(out=e[:, CA:], in_=d[:, CA:], func=AF.Exp).then_inc(
                se, 1
            )
            scalar.wait_ge(se, 2)
            nc.scalar.activation(
                out=act_out[:, CA:],
                in_=e[:, CA:],
                func=AF.Ln,
                bias=1.0,
                accum_out=part[:, 1:2],
            ).then_inc(sl, 1)

        @block.gpsimd
        def _(gpsimd):
            # dummy indirect DMA: pull the Q7 indirect-copy ucode + SWDGE ring
            # setup while the idx tiles load (the first indirect DMA
            # otherwise pays ~2.5us of cold-start). Indices come from the
            # preamble-written const-0.0 AP bitcast to int32 (= all zeros).
            gpsimd.indirect_dma_start(
                out=warm_out[:32, 0:1],
                out_offset=None,
                in_=predict[:],
                in_offset=bass.IndirectOffsetOnAxis(
                    ap=zero.bitcast(mybir.dt.int32)[:32, :], axis=0
                ),
            ).then_inc(s_warm, 16)
            # second dummy keeps the Q7 SWDGE pipeline hot until the idx
            # receipt lands: a descgen dispatched after an idle gap pays
            # ~0.9us of wake-up before its emission loop starts
            gpsimd.indirect_dma_start(
                out=warm_out[:32, 1:2],
                out_offset=None,
                in_=predict[:],
                in_offset=bass.IndirectOffsetOnAxis(
                    ap=zero.bitcast(mybir.dt.int32)[:32, :], axis=0
                ),
            ).then_inc(s_warm, 16)
            gpsimd.wait_ge(s_i1, 16)
            gpsimd.wait_ge(s_i1b, 16)
            gpsimd.indirect_dma_start(
                out=vals[:, :WA],
                out_offset=None,
                in_=predict[:],
                in_offset=bass.IndirectOffsetOnAxis(ap=idx_t[:, :WA], axis=0),
            ).then_inc(s_g1, 16)
            gpsimd.wait_ge(s_i2, 16)
            gpsimd.indirect_dma_start(
                out=vals[:, WA:],
                out_offset=None,
                in_=predict[:],
                in_offset=bass.IndirectOffsetOnAxis(ap=idx_t[:, WA:], axis=0),
            ).then_inc(s_g2, 16)

        @block.vector
        def _(vector):
            # chunk A: cols [0:CA]=pos, [CA:WA]=neg; d = neg - pos
            vector.wait_ge(s_g1, 16)
            nc.vector.tensor_tensor(
                out=d[:, :CA],
                in0=vals[:, CA:WA],
                in1=vals[:, :CA],
                op=mybir.AluOpType.subtract,
            ).then_inc(sv, 1)
            # chunk B: cols [WA:WA+CB]=pos, [WA+CB:]=neg
            vector.wait_ge(s_g2, 16)
            nc.vector.tensor_tensor(
                out=d[:, CA:],
                in0=vals[:, WA + CB :],
                in1=vals[:, WA : WA + CB],
                op=mybir.AluOpType.subtract,
            ).then_inc(sv, 1)
            vector.wait_ge(sm, 1)
            nc.vector.tensor_copy(out=scalar_out[:], in_=psum_s[:]).then_inc(sc, 1)

        @block.tensor
        def _(tensor):
            # dummy matmul warms the PE weight-load path during the DMA phase
            nc.tensor.matmul(
                out=psum_w[:], lhsT=ones, rhs=ones[:, 0:1], start=True, stop=True
            )
            # chunk A's partial reduces into PSUM while chunk B drains; after
            # ln B only the second accumulating matmul remains
            tensor.wait_ge(sl, 1)
            nc.tensor.matmul(
                out=psum_s[:, 0:1], lhsT=ones, rhs=part[:, 0:1], start=True, stop=True
            )
            tensor.wait_ge(sl, 2)
            nc.tensor.matmul(
                out=psum_s[:, 1:2], lhsT=ones, rhs=part[:, 1:2], start=True, stop=True
            ).then_inc(sm, 1)

    return nc


def make_in_maps(predict, pos_idx, neg_idx):
    predict = np.ascontiguousarray(np.asarray(predict), dtype=np.float32)
    pos_idx = np.asarray(pos_idx)
    neg_idx = np.asarray(neg_idx)

    in_maps = []
    row_off = (np.arange(RB, dtype=np.int64)[:, None] * N)  # (512, 1)
    na = P * CA  # pairs in chunk A
    for c in range(NCORES):
        r0 = c * RB
        fp = (row_off + pos_idx[r0 : r0 + RB].astype(np.int64)).reshape(-1)
        fn = (row_off + neg_idx[r0 : r0 + RB].astype(np.int64)).reshape(-1)
        # pair placement is free (the loss is a flat sum over pairs): sort by
        # pos address, give each SDMA engine a contiguous 1/16 of the span
        # (its 8 partitions, dealt round-robin) so every engine's descriptor
        # stream walks one ascending HBM window
        order = np.argsort(fp, kind="stable")
        fp = fp[order].astype(np.int32)
        fn = fn[order].astype(np.int32)

        def chunk_layout(fpc, fnc, cols):
            # fpc/fnc: (P*cols,) sorted pair span -> (P, cols) tiles
            pt = np.empty((P, cols), np.int32)
            nt = np.empty((P, cols), np.int32)
            eng_parts = [[p for p in range(P)
                          if ((p % 32) // 4) * 2 + (p // 64) == k]
                         for k in range(16)]
            per_eng = len(fpc) // 16  # 8*cols pairs per engine
            for k in range(16):
                blk_p = fpc[k * per_eng : (k + 1) * per_eng]
                blk_n = fnc[k * per_eng : (k + 1) * per_eng]
                t = np.arange(per_eng)
                rows = np.asarray(eng_parts[k])[t % 8]
                cols_i = t // 8
                pt[rows, cols_i] = blk_p
                nt[rows, cols_i] = blk_n
            return pt, nt

        pA, nA_ = chunk_layout(fp[:na], fn[:na], CA)
        pB, nB = chunk_layout(fp[na:], fn[na:], CB)
        idx_all = np.concatenate([pA, nA_, pB, nB], axis=1)  # (128, 400)
        in_maps.append(
            {
                "predict": predict[r0 : r0 + RB].reshape(-1, 1),
                "idx": np.ascontiguousarray(idx_all),
            }
        )
    return in_maps


def run(predict, pos_idx, neg_idx, trace=False, **kwargs):
    global _NC_CACHE
    if _NC_CACHE is None:
        _NC_CACHE = build_bass()
    nc = _NC_CACHE
    in_maps = make_in_maps(predict, pos_idx, neg_idx)
    res = run_bass_kernel_spmd(nc, in_maps, list(range(NCORES)), trace=trace, **kwargs)
    total = np.float64(0.0)
    for r in res.results:
        total += np.float64(r["out"].astype(np.float64).sum())
    out = np.float32(total / K)
    return out, res


def kernel(predict, pos_idx, neg_idx):
    out, _ = run(predict, pos_idx, neg_idx, trace=False)
    return out

